# revision 1
# baseline (speedup 1.0000x reference)
"""Self-contained Trainium2 Bass kernel for nn_AtomsNetwork (gnn_message_passing).

Sharding: atoms split 8 ways across the chip's NeuronCores (2000/core).
Per protein p:
  L1: neighbor-signal tables sig_s=atoms@Wsr1, sig_d=atoms@Wdr1 are built
      shard-wise on TensorE in gather-table layout (bf16), AllGathered via
      the collective engine, then neighbor means are formed with chunked
      SWDGE dma_gather (SBUF-source, transposed output) + DVE windowed
      K-reduces; y = relu(atoms@Wv + residues@Wr + means@...) accumulates
      fully in PSUM (means are injected with identity matmuls after an
      outer-product reciprocal scale).
  L2: y itself is the gather table (bf16, AllGathered); w = relu(y@Wsv +
      mean_s@Wsr2 + mean_d@Wdr2).
Residue means: one-hot segment matmul per 128-atom chunk, partial sums
AllReduced across cores, scaled by host-derived 1/count.
Head: each core computes its 50 rows of the 400x400 residue-pair grid;
row selection is input-driven (one-hot sel matrix) so the SPMD graph is
identical on every core. x@Wf1 is decomposed as A[i]+B[j] (rank trick),
relu'd via per-partition-bias activation ops, then Wf2/Wf3 matmuls.
"""
import sys
import numpy as np

sys.path.insert(0, '/opt/trn_rl_repo')

N_ATOMS = 16000
NC = 8
K = 10
N_RES = 400
ATOM_CAT = 12
BERT_DIM = 1024
DF2 = 64


def build_graph(N, R, BERT):
    from concourse import bass, bacc, mybir
    from concourse.alu_op_type import AluOpType
    f32, bf16, i16 = mybir.dt.float32, mybir.dt.bfloat16, mybir.dt.int16
    AF = mybir.ActivationFunctionType

    LOC = N // NC
    LOCP = -(-LOC // 128) * 128
    STR = LOCP // 128 + (1 if LOC == LOCP else 0)
    CH_AT = min(256, LOCP)
    CH = CH_AT * K
    NSEG = LOCP // CH_AT
    NID = 2 * LOCP * K
    KB = BERT // 128
    MYR = R // NC
    RC = -(-R // 128)
    NT = -(-LOCP // 512)
    NRT = -(-2 * R // 512)
    AC = LOCP // 128          # atom chunks of 128

    nc = bacc.Bacc()
    P = lambda n, s, d: nc.declare_dram_parameter(n, s, d, isOutput=False)
    ins = {}
    for p in (1, 2):
        ins[f'atomsT_{p}'] = P(f'atomsT_{p}', [ATOM_CAT, LOCP], bf16)
        ins[f'residT_{p}'] = P(f'residT_{p}', [KB, 128, LOCP], bf16)
        ins[f'idxL1_{p}'] = P(f'idxL1_{p}', [128, NID // 16], i16)
        ins[f'idxL2_{p}'] = P(f'idxL2_{p}', [128, NID // 16], i16)
        ins[f'recips_{p}'] = P(f'recips_{p}', [1, 4 * LOCP], f32)
        ins[f'rids_{p}'] = P(f'rids_{p}', [128, AC], f32)
    for nm, sh in [
                   ('Wsv', [128, 128]), ('Wsr2', [128, 128]), ('Wdr2', [128, 128]),
                   ('Wf1t', [128, 256]), ('Wf1b', [128, 256]),

                   ('bf1', [128, 2]), ('bf2', [DF2, 1]), ('bf3', [1, 1]),
                   ('recip_res', [1, 2 * R])]:
        ins[nm] = P(nm, sh, f32)
    ins['Wf2'] = P('Wf2', [128, 2 * DF2], bf16)
    ins['sel'] = P('sel', [128, RC * MYR], bf16)
    ins['Wf3'] = P('Wf3', [DF2, 1], bf16)
    ins['Wr'] = P('Wr', [128, KB * 128], bf16)
    for nm in ('Wv', 'Wsr1', 'Wdr1'):
        ins[nm] = P(nm, [ATOM_CAT, 128], bf16)
    out_ext = nc.declare_dram_parameter('out', [1, MYR * R], f32, isOutput=True)

    shardL1s = [nc.dram_tensor(f'shardL1_{p}', [2, STR, 128, 128], bf16) for p in (0, 1)]
    fullL1s = [nc.dram_tensor(f'fullL1_{p}', [NC, 2, STR, 128, 128], bf16, addr_space='Shared')
               for p in (0, 1)]
    shardY = nc.dram_tensor('shardY', [1, STR, 128, 128], bf16)
    fullY = [nc.dram_tensor(f'fullY{p}', [NC, 1, STR, 128, 128], bf16, addr_space='Shared')
             for p in (0, 1)]
    rparts_d = [nc.dram_tensor(f'rpart_d{p}', [128, R], f32) for p in (0, 1)]
    rsums_d = [nc.dram_tensor(f'rsum_d{p}', [128, R], f32, addr_space='Shared')
               for p in (0, 1)]

    steps = []
    cnt = {}

    class Tok:
        __slots__ = ('sem', 'n')
        def __init__(s, sem, n): s.sem, s.n = sem, n

    from contextlib import ExitStack
    _es = ExitStack()
    with _es:
        block = _es.enter_context(nc.Block())
        sem_dma = _es.enter_context(nc.semaphore('dma'))
        sem_gat1 = _es.enter_context(nc.semaphore('gat1'))
        sem_gat2 = _es.enter_context(nc.semaphore('gat2'))
        sem_dmaS = _es.enter_context(nc.semaphore('dmaS'))
        sem_dmaT = _es.enter_context(nc.semaphore('dmaT'))
        sem_dmaR0 = _es.enter_context(nc.semaphore('dmaR0'))
        sem_dmaR1 = _es.enter_context(nc.semaphore('dmaR1'))
        sem_dmaU = _es.enter_context(nc.semaphore('dmaU'))
        sem_dmaC = _es.enter_context(nc.semaphore('dmaC'))
        sem_gat = _es.enter_context(nc.semaphore('gat'))
        sem_pe = _es.enter_context(nc.semaphore('pe'))
        sem_v = _es.enter_context(nc.semaphore('v'))
        sem_act = _es.enter_context(nc.semaphore('act'))
        sem_g = _es.enter_context(nc.semaphore('g'))
        sem_cc = _es.enter_context(nc.semaphore('cc'))
        tabL1 = _es.enter_context(nc.sbuf_tensor('tabL1', [128, 2 * NC * STR * 128], bf16))
        gbuf = _es.enter_context(nc.sbuf_tensor('gbuf', [128, 3, CH], bf16))
        idxA = _es.enter_context(nc.sbuf_tensor('idxA', [128, NID // 16], i16))
        idxB = _es.enter_context(nc.sbuf_tensor('idxB', [128, NID // 16], i16))
        meanS = _es.enter_context(nc.sbuf_tensor('meanS', [128, LOCP], f32))
        meanD = _es.enter_context(nc.sbuf_tensor('meanD', [128, LOCP], f32))
        ybuf = _es.enter_context(nc.sbuf_tensor('ybuf', [128, 2 * LOCP], f32))
        wbuf = _es.enter_context(nc.sbuf_tensor('wbuf', [128, LOCP], f32))
        rows16 = _es.enter_context(nc.sbuf_tensor('rows16', [128, 2, 128], bf16))
        rowsR = _es.enter_context(nc.sbuf_tensor('rowsR', [128, RC, 128], bf16))
        stripes = _es.enter_context(nc.sbuf_tensor('stripes', [128, 2, STR, 128], bf16))
        resb = _es.enter_context(nc.sbuf_tensor('resb', [128, 2 * LOCP], bf16))
        Mbuf = _es.enter_context(nc.sbuf_tensor('Mbuf', [128, 2 * R], bf16))
        atomsT = _es.enter_context(nc.sbuf_tensor('atomsT', [ATOM_CAT, 2 * LOCP], bf16))
        rcpbuf = _es.enter_context(nc.sbuf_tensor('rcpbuf', [1, LOCP], f32))
        ridsb = _es.enter_context(nc.sbuf_tensor('ridsb', [128, 2 * AC], f32))
        rbuf = _es.enter_context(nc.sbuf_tensor('rbuf', [128, 2 * R], f32))
        rT = _es.enter_context(nc.sbuf_tensor('rT', [128, 2 * R], f32))
        r1my = _es.enter_context(nc.sbuf_tensor('r1my', [128, MYR], f32))
        Abuf = _es.enter_context(nc.sbuf_tensor('Abuf', [128, 2 * MYR], f32))
        Bbuf = _es.enter_context(nc.sbuf_tensor('Bbuf', [128, 2 * R], bf16))
        Xbuf = _es.enter_context(nc.sbuf_tensor('Xbuf', [128, 2 * R], bf16))
        h2b = _es.enter_context(nc.sbuf_tensor('h2b', [DF2, R], bf16))
        outb = _es.enter_context(nc.sbuf_tensor('outb', [1, 2 * R], f32))
        iotaP = _es.enter_context(nc.sbuf_tensor('iotaP', [128, 128], f32))
        iotaR = _es.enter_context(nc.sbuf_tensor('iotaR', [128, R], f32))
        ones1 = _es.enter_context(nc.sbuf_tensor('ones1', [1, 128], f32))
        wWv = _es.enter_context(nc.sbuf_tensor('wWv', [ATOM_CAT, 128], bf16))
        wWr = _es.enter_context(nc.sbuf_tensor('wWr', [128, KB * 128], bf16))
        wWsr1 = _es.enter_context(nc.sbuf_tensor('wWsr1', [ATOM_CAT, 128], bf16))
        wWdr1 = _es.enter_context(nc.sbuf_tensor('wWdr1', [ATOM_CAT, 128], bf16))
        wWsv = _es.enter_context(nc.sbuf_tensor('wWsv', [128, 128], f32))
        wWsr2 = _es.enter_context(nc.sbuf_tensor('wWsr2', [128, 128], f32))
        wWdr2 = _es.enter_context(nc.sbuf_tensor('wWdr2', [128, 128], f32))
        wWf1t = _es.enter_context(nc.sbuf_tensor('wWf1t', [128, 256], f32))
        wWf1b = _es.enter_context(nc.sbuf_tensor('wWf1b', [128, 256], f32))
        wWf2 = _es.enter_context(nc.sbuf_tensor('wWf2', [128, 2 * DF2], bf16))
        wWf3 = _es.enter_context(nc.sbuf_tensor('wWf3', [DF2, 1], bf16))
        wbf1 = _es.enter_context(nc.sbuf_tensor('wbf1', [128, 2], f32))
        wbf2 = _es.enter_context(nc.sbuf_tensor('wbf2', [DF2, 1], f32))
        wbf3 = _es.enter_context(nc.sbuf_tensor('wbf3', [1, 1], f32))
        wrr = _es.enter_context(nc.sbuf_tensor('wrr', [1, 2 * R], f32))
        wsel = _es.enter_context(nc.sbuf_tensor('wsel', [128, RC * MYR], bf16))
        sems = {'dma': sem_dma, 'gat': sem_gat, 'pe': sem_pe, 'v': sem_v,
                'act': sem_act, 'g': sem_g, 'cc': sem_cc,
                'gat1': sem_gat1, 'gat2': sem_gat2, 'dmaS': sem_dmaS, 'dmaT': sem_dmaT, 'dmaR0': sem_dmaR0,
                'dmaR1': sem_dmaR1, 'dmaU': sem_dmaU, 'dmaC': sem_dmaC}

        def S(eng, emit, waits=(), inc=None, amt=1):
            _m = {}
            for t in waits:
                if t is not None and _m.get(id(t.sem), (None, -1))[1] < t.n:
                    _m[id(t.sem)] = (t.sem, t.n)
            cw = list(_m.values())
            semobj = sems[inc] if inc else None
            def fn(e, cw=cw, emit=emit, semobj=semobj, amt=amt):
                for sm, n in cw:
                    e.wait_ge(sm, n)
                r = emit(e)
                if semobj is not None:
                    r.then_inc(semobj, amt)
            steps.append((eng, fn))
            if inc:
                cnt[inc] = cnt.get(inc, 0) + amt
                return Tok(sems[inc], cnt[inc])
            return None

        zpsum = nc.place_psum_tensor('zps', [128, LOCP], f32, bank=0)
        rcpps = nc.place_psum_tensor('rcp', [128, LOCP], f32, bank=4)
        trps = [nc.place_psum_tensor(f'tr{i}', [128, 128], f32, bank=i) for i in (0, 1)]
        segps = nc.place_psum_tensor('seg', [128, R], f32, bank=2)
        r1ps = nc.place_psum_tensor('r1p', [128, MYR], f32, bank=3)
        rrps = nc.place_psum_tensor('rrp', [128, 2 * R], f32, bank=4)
        Bps = [nc.place_psum_tensor(f'Bp{i}', [128, R], f32, bank=6 + i) for i in (0, 1)]
        h2ps = [nc.place_psum_tensor(f'h2p{i}', [DF2, R], f32, bank=4 + i) for i in (0, 1)]
        h3ps = [nc.place_psum_tensor(f'h3p{i}', [1, R], f32, bank=2 + i) for i in (0, 1)]

        D = lambda out, in_: (lambda e: e.dma_start(out=out, in_=in_))

        # ---------- phase 0: constants + input loads ----------
        t_dma = None
        for nm, dst in [('Wv', wWv), ('Wr', wWr), ('Wsr1', wWsr1), ('Wdr1', wWdr1),
                        ('Wsv', wWsv), ('Wsr2', wWsr2), ('Wdr2', wWdr2),
                        ('Wf1t', wWf1t), ('Wf1b', wWf1b), ('Wf2', wWf2),
                        ('Wf3', wWf3), ('bf1', wbf1), ('bf2', wbf2), ('bf3', wbf3),
                        ('recip_res', wrr), ('sel', wsel)]:
            t_dma = S('sync', D(dst[:], ins[nm][:]), inc='dma', amt=16)
        for p in (1, 2):
            t_dma = S('sync', D(atomsT[:, (p - 1) * LOCP:p * LOCP], ins[f'atomsT_{p}'][:]),
                      inc='dma', amt=16)
            t_dma = S('sync', D(ridsb[:, (p - 1) * AC:p * AC], ins[f'rids_{p}'][:]),
                      inc='dma', amt=16)

        t_io = S('g', lambda e: e.iota(iotaP[:], [[1, 128]], channel_multiplier=-1,
                                       allow_small_or_imprecise_dtypes=True), inc='g')
        t_id = S('v', lambda e: e.tensor_scalar(out=iotaP[:], in0=iotaP[:], scalar1=0.0,
                                                scalar2=None, op0=AluOpType.is_equal),
                 waits=[t_io], inc='v')
        t_ir = S('g', lambda e: e.iota(iotaR[:], [[1, R]], channel_multiplier=0,
                                       allow_small_or_imprecise_dtypes=True),
                 waits=[t_io], inc='g')
        t_ones = S('v', lambda e: e.memset(ones1[:], 1.0), inc='v')

        state = {'last_reds': [None, None, None], 'last_gat': [None, None, None],
                 'stripes_free': None, 'idx_free': [], 'tr': [None, None], 'rcp_free': None}

        def gathers_and_means(idx_sb, tok_table, tok_idx, tab_elems, pnum, layer, tab_off=0):
            toks_red = []
            tok_all_dmaT = Tok(sems['dmaT'], cnt.get('dmaT', 0))
            for c in range(2 * NSEG):
                side, q, buf = c // NSEG, c % NSEG, c % 3
                w = [tok_all_dmaT, state['last_reds'][buf]]
                tg = S('g', (lambda e, c=c, buf=buf, idx_sb=idx_sb, tab_elems=tab_elems:
                             e.dma_gather(
                                 out_ap=gbuf[:, buf, :].unsqueeze(1),
                                 in_ap=tabL1[:, tab_off:tab_off + tab_elems],
                                 idxs_ap=idx_sb[:, c * (CH // 16):(c + 1) * (CH // 16)],
                                 num_idxs=CH, num_idxs_reg=CH,
                                 elem_size=128, transpose=True,
                                 sbuf_tokens_per_rank=128,
                                 sbuf_free_dim_per_rank=256,
                                 sbuf_free_dim_pad_per_rank=0,
                                 sbuf_byte_offset=0,
                                 single_packet=False)),
                        waits=w, inc=('gat', 'gat1', 'gat2')[buf], amt=16)
                dst = meanS if side == 0 else meanD
                tr = S('v', (lambda e, dst=dst, q=q, buf=buf:
                             e.tensor_reduce(
                                 dst[:, q * CH_AT:(q + 1) * CH_AT],
                                 gbuf[:, buf, :].rearrange('p (a k) -> p a k', k=K),
                                 mybir.AxisListType.X, AluOpType.add)),
                       waits=[tg], inc='v')
                toks_red.append(tr)
                state['last_reds'][buf] = tr
                state['last_gat'] = state['last_gat'][1:] + [tg]
            # recip scaling
            sc_toks = []
            for side, dst, tokr in ((0, meanS, toks_red[NSEG - 1]),
                                    (1, meanD, toks_red[2 * NSEG - 1])):
                off_d = (layer * 2 + side) * LOCP
                trc = S('sync', (lambda e, off_d=off_d, pnum=pnum:
                                 e.dma_start(out=rcpbuf[:],
                                             in_=ins[f'recips_{pnum}'][:, off_d:off_d + LOCP])),
                        waits=[state['rcp_free']], inc='dmaC', amt=16)
                tks = []
                for nt in range(NT):
                    n0, n1 = nt * 512, min((nt + 1) * 512, LOCP)
                    tks.append(S('pe', (lambda e, n0=n0, n1=n1:
                                        e.matmul(rcpps[:, n0:n1], ones1[:],
                                                 rcpbuf[:, n0:n1],
                                                 start=True, stop=True)),
                                 waits=[tokr, t_ones, trc] +
                                       ([sc_toks[-1]] if sc_toks else []), inc='pe'))
                state['rcp_free'] = tks[-1]
                sc_toks.append(S('v', (lambda e, dst=dst:
                                       e.tensor_tensor(out=dst[:], in0=dst[:],
                                                       in1=rcpps[:], op=AluOpType.mult)),
                                 waits=tks + [tokr], inc='v'))
            return sc_toks

        def emit_rows(src_ap_fn, n_chunks, dst_copy_fn, tok_src, extra_first_wait=None):
            """PE-transpose n_chunks [128,128] blocks of src, copy each out."""
            toks = []
            for c in range(n_chunks):
                tps = trps[c % 2]
                w = [tok_src, t_id, state['tr'][c % 2]]
                if c == 0 and extra_first_wait is not None:
                    w.append(extra_first_wait)
                tk = S('pe', (lambda e, tps=tps, c=c:
                              (lambda sap: e.transpose(tps[0:sap.shape[-1], :], sap,
                                                       iotaP[:]))(src_ap_fn(c))),
                       waits=w, inc='pe')
                tc = S('v', (lambda e, tps=tps, c=c: dst_copy_fn(e, tps, c)),
                       waits=[tk], inc='v')
                state['tr'][c % 2] = tc
                toks.append(tc)
            return toks

        def pad_stripes(tab_list, tok_after):
            # pad slots [LOC, LOCP) are structurally zero (host zero-pads inputs);
            # only the extra all-zero stripe (if any) needs an explicit memset.
            toks = [tok_after]
            if STR > AC:
                for tb in tab_list:
                    toks.append(S('v', (lambda e, tb=tb:
                                        e.memset(stripes[:, tb, STR - 1, :], 0.0)),
                                  waits=[toks[-1]], inc='v'))
            return toks[-1]

        # build BOTH proteins' sig tables and launch their AllGathers up front
        sig_cc = []
        for p in (0, 1):
            aT0 = p * LOCP
            toks_stripe = []
            for tab, W in ((0, wWsr1), (1, wWdr1)):
                for c in range(AC):
                    buf = (tab * AC + c) % 2
                    tps = trps[buf]
                    w = [t_dma, state['tr'][buf]]
                    if len(toks_stripe) == 0 and state['stripes_free'] is not None:
                        w.append(state['stripes_free'])
                    tk = S('pe', (lambda e, tps=tps, c=c, W=W, aT0=aT0:
                                  e.matmul(tps[:], atomsT[:, aT0 + c * 128:aT0 + (c + 1) * 128],
                                           W[:], start=True, stop=True)),
                           waits=w, inc='pe')
                    tc = S('v', (lambda e, tps=tps, tab=tab, c=c:
                                 e.tensor_copy(stripes[:, tab, c, :], tps[:])),
                           waits=[tk] + ([state['stripes_free']]
                                         if state['stripes_free'] else []),
                           inc='v')
                    state['tr'][buf] = tc
                    toks_stripe.append(tc)
            tz = pad_stripes([0, 1], toks_stripe[-1])
            tsh = S('sync', (lambda e, p=p: e.dma_start(
                        out=shardL1s[p][:].rearrange('t s p e -> p t s e'),
                        in_=stripes[:])),
                    waits=[tz, toks_stripe[-1]], inc='dmaS', amt=16)
            sig_cc.append(S('g', (lambda e, p=p: e.collective_compute(
                        'AllGather', mybir.AluOpType.bypass,
                        replica_groups=[list(range(NC))],
                        ins=[shardL1s[p][:]], outs=[fullL1s[p][:]])),
                    waits=[tsh], inc='cc'))
            state['stripes_free'] = tsh

        tokens_y = []
        y_cc = []
        for p in (0, 1):
            aT0 = p * LOCP
            ttab = S('sync', (lambda e, p=p: e.dma_start(
                        out=tabL1[:].rearrange('p (r t s e) -> p r t s e',
                                               r=NC, t=2, s=STR),
                        in_=fullL1s[p][:].rearrange('r t s p e -> p r t s e'))),
                     waits=[sig_cc[p]] + state['last_gat'], inc='dmaT', amt=16)
            tidx = S('sync', D(idxA[:], ins[f'idxL1_{p + 1}'][:]),
                     waits=(state['idx_free'] or []), inc='dmaT', amt=16)

            sc_toks = gathers_and_means(idxA, ttab, tidx, 2 * NC * STR * 128,
                                        p + 1, 0)
            state['idx_free'] = list(state['last_gat'])

            # --- y = relu(Wv + means + Wr-stream) ---
            tmm = None
            for nt in range(NT):
                n0, n1 = nt * 512, min((nt + 1) * 512, LOCP)
                tmm = S('pe', (lambda e, n0=n0, n1=n1, aT0=aT0:
                               e.matmul(zpsum[:, n0:n1], wWv[:],
                                        atomsT[:, aT0 + n0:aT0 + n1],
                                        start=True, stop=False)),
                        waits=[t_dma] + sc_toks, inc='pe')
                tmm = S('pe', (lambda e, n0=n0, n1=n1:
                               e.matmul(zpsum[:, n0:n1], iotaP[:], meanS[:, n0:n1],
                                        start=False, stop=False)),
                        waits=[t_id], inc='pe')
                tmm = S('pe', (lambda e, n0=n0, n1=n1:
                               e.matmul(zpsum[:, n0:n1], iotaP[:], meanD[:, n0:n1],
                                        start=False, stop=False)), inc='pe')
            tres_prev = [None, None]
            for kb in range(KB):
                buf = kb % 2
                trd = S('sync', D(resb[:, buf * LOCP:(buf + 1) * LOCP],
                                  ins[f'residT_{p + 1}'][kb]),
                        waits=[tres_prev[0]], inc=f'dmaR{buf}', amt=16)
                lmm = None
                for nt in range(NT):
                    n0, n1 = nt * 512, min((nt + 1) * 512, LOCP)
                    lmm = S('pe', (lambda e, kb=kb, n0=n0, n1=n1, buf=buf:
                                   e.matmul(zpsum[:, n0:n1],
                                            wWr[:, kb * 128:(kb + 1) * 128],
                                            resb[:, buf * LOCP + n0:buf * LOCP + n1],
                                            start=False, stop=(kb == KB - 1))),
                            waits=[trd], inc='pe')
                tres_prev = [tres_prev[1], lmm]
            t_y = S('act', (lambda e, p=p: e.activation(
                        ybuf[:, p * LOCP:(p + 1) * LOCP], zpsum[:], AF.Relu)),
                    waits=[tres_prev[1]], inc='act')
            tokens_y.append(t_y)

            # --- y rows -> stripes[:,0] -> AllGather ---
            rows_toks = emit_rows(
                (lambda c, p=p: ybuf[:, p * LOCP + c * 128:p * LOCP + (c + 1) * 128]),
                AC,
                (lambda e, tps, c: e.tensor_copy(stripes[:, 0, c, :], tps[:])),
                t_y, extra_first_wait=tsh)
            tz = pad_stripes([0], rows_toks[-1])
            tshy = S('sync', D(shardY[:].rearrange('t s p e -> p t s e'),
                               stripes[:, 0:1, :, :]),
                     waits=[tz, rows_toks[-1]], inc='dmaS', amt=16)
            y_cc.append(S('g', (lambda e, p=p: e.collective_compute(
                        'AllGather', mybir.AluOpType.bypass,
                        replica_groups=[list(range(NC))],
                        ins=[shardY[:]], outs=[fullY[p][:]])),
              waits=[tshy], inc='cc'))
            state['stripes_free'] = tshy

        def emit_rexchange(p, tok_r):
            tup = S('sync', (lambda e, p=p: e.dma_start(
                        out=rparts_d[p][:], in_=rbuf[:, p * R:(p + 1) * R])),
                    waits=[tok_r], inc='dmaU', amt=16)
            tcc = S('g', (lambda e, p=p: e.collective_compute(
                        'AllReduce', mybir.AluOpType.add,
                        replica_groups=[list(range(NC))],
                        ins=[rparts_d[p][:]], outs=[rsums_d[p][:]])),
                    waits=[tup], inc='cc')
            tdn = S('sync', (lambda e, p=p: e.dma_start(
                        out=rbuf[:, p * R:(p + 1) * R], in_=rsums_d[p][:])),
                    waits=[tcc], inc='dmaU', amt=16)
            trr = S('pe', (lambda e, p=p: e.matmul(rrps[:, 0:R], ones1[:],
                        wrr[:, p * R:(p + 1) * R], start=True, stop=True)),
                    waits=[t_ones, t_dma, tdn, state.get('rr_free')], inc='pe')
            tm = S('v', (lambda e, p=p: e.tensor_tensor(
                        out=rT[:, p * R:(p + 1) * R],
                        in0=rbuf[:, p * R:(p + 1) * R], in1=rrps[:, 0:R],
                        op=AluOpType.mult)),
                   waits=[trr, tdn], inc='v')
            state['rr_free'] = tm
            return tm

        # ---------- L2 ----------
        # snapshot of L1p2's last gathers: frees the table region + idxA
        l1_gat_done = list(state['last_gat'])
        t_r = []
        YT = NC * STR * 128
        for p in (0, 1):
            off = p * YT          # p1 -> first half, p2 -> second half (preloaded)
            idxbuf = idxB if p == 0 else idxA
            ttab = S('sync', (lambda e, p=p, off=off: e.dma_start(
                        out=tabL1[:, off:off + YT].rearrange(
                            'p (r t s e) -> p r t s e', r=NC, t=1, s=STR),
                        in_=fullY[p][:].rearrange('r t s p e -> p r t s e'))),
                     waits=[y_cc[p]] + (state['last_gat'] if p == 0 else l1_gat_done),
                     inc='dmaT', amt=16)
            tidx = S('sync', (lambda e, p=p, idxbuf=idxbuf: e.dma_start(
                        out=idxbuf[:], in_=ins[f'idxL2_{p + 1}'][:])),
                     waits=(state['idx_free'] if p == 0 else l1_gat_done) or [],
                     inc='dmaT', amt=16)
            sc_toks = gathers_and_means(idxbuf, ttab, tidx, YT,
                                        p + 1, 1, tab_off=off)
            state['idx_free'] = list(state['last_gat'])

            lmm = None
            for nt in range(NT):
                n0, n1 = nt * 512, min((nt + 1) * 512, LOCP)
                lmm = S('pe', (lambda e, n0=n0, n1=n1, p=p:
                               e.matmul(zpsum[:, n0:n1], wWsv[:],
                                        ybuf[:, p * LOCP + n0:p * LOCP + n1],
                                        start=True, stop=False)),
                        waits=[tokens_y[p]] + sc_toks, inc='pe')
                lmm = S('pe', (lambda e, n0=n0, n1=n1:
                               e.matmul(zpsum[:, n0:n1], wWsr2[:], meanS[:, n0:n1],
                                        start=False, stop=False)), inc='pe')
                lmm = S('pe', (lambda e, n0=n0, n1=n1:
                               e.matmul(zpsum[:, n0:n1], wWdr2[:], meanD[:, n0:n1],
                                        start=False, stop=True)), inc='pe')
            t_w = S('act', lambda e: e.activation(wbuf[:], zpsum[:], AF.Relu),
                    waits=[lmm], inc='act')

            # fused per-chunk: transpose w rows -> rows16 ping-pong; M ping-pong; seg matmul
            tseg = None
            segs = []
            for c in range(AC):
                mb = c % 2
                tpsb = trps[mb]
                wtr = [t_w, t_id, state['tr'][mb]]
                if len(segs) >= 2:
                    wtr.append(segs[-2])
                tk = S('pe', (lambda e, tpsb=tpsb, c=c:
                              e.transpose(tpsb[:], wbuf[:, c * 128:(c + 1) * 128],
                                          iotaP[:])),
                       waits=wtr, inc='pe')
                trow = S('v', (lambda e, tpsb=tpsb, mb=mb:
                               e.tensor_copy(rows16[:, mb, :], tpsb[:])),
                         waits=[tk] + ([segs[-2]] if len(segs) >= 2 else []), inc='v')
                state['tr'][mb] = trow
                tM = S('v', (lambda e, c=c, p=p, mb=mb:
                             e.tensor_scalar(out=Mbuf[:, mb * R:(mb + 1) * R],
                                             in0=iotaR[:],
                                             scalar1=ridsb[:, p * AC + c:p * AC + c + 1],
                                             scalar2=None,
                                             op0=AluOpType.is_equal)),
                       waits=[t_ir, t_dma] + ([segs[-2]] if len(segs) >= 2 else []),
                       inc='v')
                tseg = S('pe', (lambda e, mb=mb, c=c:
                                e.matmul(segps[:], rows16[:, mb, :],
                                         Mbuf[:, mb * R:(mb + 1) * R],
                                         start=(c == 0), stop=(c == AC - 1))),
                         waits=[trow, tM], inc='pe')
                segs.append(tseg)
            t_r.append(S('v', (lambda e, p=p:
                               e.tensor_copy(rbuf[:, p * R:(p + 1) * R], segps[:])),
                         waits=[tseg], inc='v'))
            if p == 0:
                t_rT1 = emit_rexchange(0, t_r[0])
        t_rT2 = emit_rexchange(1, t_r[1])

        # ---------- head ----------
        # r1 row blocks [res,feat] then select my rows: r1T_my [128f, MYR]
        def r1rows_src(c):
            n0, n1 = c * 128, min((c + 1) * 128, R)
            return rT[:, n0:n1]
        rowsel = emit_rows(r1rows_src, RC,
                           (lambda e, tps, c: e.tensor_copy(
                               rowsR[:, c, :], tps[:])),
                           t_rT1)
        tsel = None
        for c in range(RC):
            nres = min((c + 1) * 128, R) - c * 128
            tsel = S('pe', (lambda e, c=c, nres=nres:
                            e.matmul(r1ps[:], rowsR[0:nres, c, :],
                                     wsel[0:nres, c * MYR:(c + 1) * MYR],
                                     start=(c == 0), stop=(c == RC - 1))),
                     waits=[rowsel[c], t_dma], inc='pe')
        t_r1my = S('v', lambda e: e.tensor_copy(r1my[:], r1ps[:]), waits=[tsel], inc='v')

        tA, tB = [], []
        for h in (0, 1):
            tk = S('pe', (lambda e, h=h:
                          e.matmul(Bps[h][:, 0:MYR], wWf1t[:, h * 128:(h + 1) * 128],
                                   r1my[:], start=True, stop=True)),
                   waits=[t_r1my, t_dma], inc='pe')
            tA.append(S('v', (lambda e, h=h: e.tensor_scalar(
                            out=Abuf[:, h * MYR:(h + 1) * MYR], in0=Bps[h][:, 0:MYR],
                            scalar1=wbf1[:, h:h + 1], scalar2=None,
                            op0=AluOpType.add)),
                        waits=[tk], inc='v'))
        for h in (0, 1):
            tk = S('pe', (lambda e, h=h:
                          e.matmul(Bps[h][:], wWf1b[:, h * 128:(h + 1) * 128],
                                   rT[:, R:2 * R], start=True, stop=True)),
                   waits=[t_rT2] + tA, inc='pe')
            tB.append(S('v', (lambda e, h=h: e.tensor_copy(
                            Bbuf[:, h * R:(h + 1) * R], Bps[h][:])),
                        waits=[tk], inc='v'))

        t_prev = [None, None]
        for i in range(MYR):
            pb = i % 2
            tx0 = S('v', (lambda e, i=i: e.tensor_scalar(
                        out=Xbuf[:, 0:R], in0=Bbuf[:, 0:R],
                        scalar1=Abuf[:, i:i + 1],
                        scalar2=0.0, op0=AluOpType.add, op1=AluOpType.max)),
                    waits=[tB[0], tA[0], t_prev[1]], inc='v')
            tx1 = S('act', (lambda e, i=i: e.activation(
                        Xbuf[:, R:2 * R], Bbuf[:, R:2 * R], AF.Relu,
                        bias=Abuf[:, MYR + i:MYR + i + 1])),
                    waits=[tB[1], tA[1], t_prev[1]], inc='act')
            tm = S('pe', (lambda e, pb=pb: e.matmul(h2ps[pb][:], wWf2[:, 0:DF2],
                                                    Xbuf[:, 0:R],
                                                    start=True, stop=False)),
                   waits=[tx0], inc='pe')
            tm = S('pe', (lambda e, pb=pb: e.matmul(h2ps[pb][:], wWf2[:, DF2:2 * DF2],
                                                    Xbuf[:, R:2 * R],
                                                    start=False, stop=True)),
                   waits=[tx1], inc='pe')
            th2 = S('act', (lambda e, pb=pb: e.activation(h2b[:], h2ps[pb][:],
                                                          AF.Relu, bias=wbf2[:])),
                    waits=[tm, t_prev[1]], inc='act')
            tm3 = S('pe', (lambda e, pb=pb: e.matmul(h3ps[pb][:], wWf3[:], h2b[:],
                                                     start=True, stop=True)),
                    waits=[th2], inc='pe')
            t_out = S('v', (lambda e, i=i, pb=pb: e.tensor_scalar(
                          out=outb[:, pb * R:(pb + 1) * R], in0=h3ps[pb][:],
                          scalar1=wbf3[:], scalar2=None, op0=AluOpType.add)),
                      waits=[tm3, state.get('outdma', [None, None])[pb]], inc='v')
            todma = S('sync', D(out_ext[:, i * R:(i + 1) * R],
                                outb[:, pb * R:(pb + 1) * R]),
                      waits=[t_out], inc=('dmaR0' if pb == 0 else 'dmaR1'), amt=16)
            od = state.get('outdma', [None, None])
            od[pb] = todma
            state['outdma'] = od
            t_prev = [t_out, tm3]

        S('sync', lambda e: e.nop(), waits=state['outdma'])

        @block.sync
        def _(e):
            for eng, fn in steps:
                if eng == 'sync':
                    fn(e)

        @block.tensor
        def _(e):
            for eng, fn in steps:
                if eng == 'pe':
                    fn(e)

        @block.vector
        def _(e):
            for eng, fn in steps:
                if eng == 'v':
                    fn(e)

        @block.scalar
        def _(e):
            for eng, fn in steps:
                if eng == 'act':
                    fn(e)

        @block.gpsimd
        def _(e):
            for eng, fn in steps:
                if eng == 'g':
                    fn(e)

    nc.finalize()
    return nc


def _bf(x):
    import ml_dtypes
    return np.asarray(x, np.float32).astype(ml_dtypes.bfloat16)


def prep_inputs(inputs, N, R, BERT):
    LOC = N // NC
    LOCP = -(-LOC // 128) * 128
    STR = LOCP // 128 + (1 if LOC == LOCP else 0)
    KB = BERT // 128
    MYR = R // NC
    RC = -(-R // 128)
    AC = LOCP // 128
    f32 = np.float32
    NID = 2 * LOCP * K

    zero_loc = LOC % 128 + (LOC // 128) * 128 if LOC < LOCP else (STR - 1) * 128

    def wrap_idx(flat):
        nid = flat.shape[0]
        w = flat.reshape(nid // 16, 16).T.astype(np.int16)
        return np.tile(w, (8, 1))

    def mk_idx(same, diff, core, layer):
        lo = core * LOC
        parts = []
        for tab, idx in ((0, same), (1, diff)):
            sl = np.asarray(idx)[lo:lo + LOC].astype(np.int64)
            rank, locl = sl // LOC, sl % LOC
            if layer == 0:
                base = (rank * 2 + tab) * STR * 128
                zs = tab * STR * 128 + zero_loc
            else:
                base = rank * STR * 128
                zs = zero_loc
            s = np.where(sl < 0, zs, base + locl)
            pad = np.full((LOCP - LOC, K), zs, np.int64)
            s = np.concatenate([s, pad], 0).reshape(-1)
            parts.append(s)
        flat = np.concatenate(parts)
        assert flat.max() < 32768, flat.max()
        return wrap_idx(flat)

    def mk_recips(same, diff, core):
        lo = core * LOC
        out = np.zeros((2, LOCP), f32)
        for t, idx in ((0, same), (1, diff)):
            m = (np.asarray(idx)[lo:lo + LOC] > -1).sum(1)
            out[t, :LOC] = 1.0 / np.maximum(m, 1)
        return out

    cnt_res = [np.zeros(R, f32), np.zeros(R, f32)]
    for p, rid in ((0, inputs['res_ids1']), (1, inputs['res_ids2'])):
        ids, c = np.unique(np.asarray(rid), return_counts=True)
        cnt_res[p][ids.astype(int)] = c
    recip_res = np.concatenate([1.0 / np.maximum(cnt_res[0], 1),
                                1.0 / np.maximum(cnt_res[1], 1)]).reshape(1, 2 * R).astype(f32)

    Wf1 = np.asarray(inputs['Wf1'], f32)
    Wf2 = np.asarray(inputs['Wf2'], f32)
    shared = {
        'Wv': _bf(inputs['Wv']),
        'Wr': _bf(np.asarray(inputs['Wr'], f32).reshape(KB, 128, 128).transpose(1, 0, 2).reshape(128, KB * 128)),
        'Wsr1': _bf(inputs['Wsr1']), 'Wdr1': _bf(inputs['Wdr1']),
        'Wsv': np.asarray(inputs['Wsv'], f32), 'Wsr2': np.asarray(inputs['Wsr2'], f32),
        'Wdr2': np.asarray(inputs['Wdr2'], f32),
        'Wf1t': Wf1[:128, :], 'Wf1b': Wf1[128:, :],
        'Wf2': _bf(np.concatenate([Wf2[:128], Wf2[128:]], axis=1)),
        'Wf3': _bf(np.asarray(inputs['Wf3'], f32).reshape(DF2, 1)),
        'bf1': np.asarray(inputs['bf1'], f32).reshape(2, 128).T.copy(),
        'bf2': np.asarray(inputs['bf2'], f32).reshape(DF2, 1),
        'bf3': np.asarray(inputs['bf3'], f32).reshape(1, 1),
        'recip_res': recip_res,
    }
    per_core = []
    for core in range(NC):
        m = dict(shared)
        lo = core * LOC
        for p, (a, r, s, d, rid) in enumerate((
                ('atoms1', 'residues1', 'same1', 'diff1', 'res_ids1'),
                ('atoms2', 'residues2', 'same2', 'diff2', 'res_ids2'))):
            at = np.zeros((ATOM_CAT, LOCP), f32)
            at[:, :LOC] = np.asarray(inputs[a], f32)[lo:lo + LOC].T
            m[f'atomsT_{p + 1}'] = _bf(at)
            rt = np.zeros((BERT, LOCP), f32)
            rt[:, :LOC] = np.asarray(inputs[r], f32)[lo:lo + LOC].T
            m[f'residT_{p + 1}'] = _bf(rt.reshape(KB, 128, LOCP))
            m[f'idxL1_{p + 1}'] = mk_idx(inputs[s], inputs[d], core, 0)
            m[f'idxL2_{p + 1}'] = mk_idx(inputs[s], inputs[d], core, 1)
            rc = mk_recips(inputs[s], inputs[d], core)
            m[f'recips_{p + 1}'] = np.concatenate([rc, rc], 0).reshape(1, 4 * LOCP)
            rr = np.full((LOCP,), -1.0, f32)
            rr[:LOC] = np.asarray(inputs[rid], f32)[lo:lo + LOC]
            m[f'rids_{p + 1}'] = rr.reshape(AC, 128).T.copy()
        sel = np.zeros((128, RC * MYR), f32)
        for j in range(MYR):
            g = core * MYR + j
            sel[g % 128, (g // 128) * MYR + j] = 1.0
        m['sel'] = _bf(sel)
        per_core.append(m)
    return per_core


def kernel(**inputs):
    from concourse.bass_utils import run_bass_kernel_spmd
    nc = build_graph(N_ATOMS, N_RES, BERT_DIM)
    in_maps = prep_inputs(inputs, N_ATOMS, N_RES, BERT_DIM)
    res = run_bass_kernel_spmd(nc, in_maps, list(range(NC)))
    out = np.concatenate([np.asarray(res.results[c]['out']).reshape(-1) for c in range(NC)])
    return out.astype(np.float32)

